# revision 29
# baseline (speedup 1.0000x reference)
"""Trainium2 Bass kernel for nn_Block_90726889161490 (sparse_attention).

Reference computation (B=4, T=2048, HIDDEN=1024, 16 heads x 64):
    LayerNorm -> fused qkvp projection (7*HIDDEN cols) -> identity seq
    "compression" (scale 1.0) -> rotary(q, k) -> full softmax attention ->
    GELU side branch on p -> concat([o, p]) @ w_out + b_out.

Sharding: 8 cores = 4 batches x 2 head-groups (tensor parallel over heads
for q/k/v/attention, column split of in_proj, row split of out_proj).
Each core computes a partial [T, HIDDEN] output; host sums the two
head-group partials per batch (the all-reduce after out_proj).

Per-core schedule (three ACT-table-clean phases):
  P1: LN (bn_stats+Sqrt) -> xn_dram[ks][T,128] -> contiguous XBAR
      transposes -> xnT; qkv psum chains; rotary on SBUF bf16 (4 DVE ops)
      -> q_dram/k_dram[hc][T,128] -> qT/kT transposes; v -> v_aug
      (65th col ones for the softmax denominator).
  P2: p projection in [pcol, tok] layout (w_p stationary, reused across
      token chunks) + exact GELU -> pt_dram.
  P3: per 512-token i-window: per head-pair, per j-chunk: row-tiled QK^T
      (2 heads concurrent on PE row groups 0-1/2-3), one fused exp over
      both heads' scores [128,1024], A^T V psum chains with the ones
      column giving denominators; out_proj MMs of the previous window
      interleaved into the PE stream to hide ACT exp time; normalize
      via reciprocal + gpsimd partition broadcast -> oT.
  P4: out_proj tail for the last window.
"""

import os
import sys

for _p in ("/opt/trn_rl_repo", "/root/.axon_site/_ro/trn_rl_repo"):
    if os.path.isdir(_p) and _p not in sys.path:
        sys.path.insert(0, _p)

import numpy as np
import ml_dtypes

import concourse.bass as bass
import concourse.mybir as mybir
import concourse.tile as tile
from concourse import bacc
from concourse.bass_utils import run_bass_kernel_spmd

F32 = mybir.dt.float32
BF16 = mybir.dt.bfloat16
AF = mybir.ActivationFunctionType
ALU = mybir.AluOpType

N_CORES = 8
B, T, HIDDEN = 4, 2048, 1024
HEADS, HEAD_DIM = 16, 64
HG = HEADS // 2          # heads per core = 8
QK = HG * HEAD_DIM       # q/k/v col-slice per core = 512
PCOLS = 4 * HIDDEN // 2  # p col-slice per core = 2048
KO = HIDDEN // 128       # 8 contraction subtiles for d=1024
TT = T // 128            # 16 token tiles
NIC = 4                  # attention i-windows of 512 tokens
JC = T // 128            # 16 attention j-chunks
LN_EPS = 1e-5


def _build_nc(trivial_ln, debug=False):
    nc = bacc.Bacc("TRN2", target_bir_lowering=False, debug=False)

    x = nc.dram_tensor("x", [T, HIDDEN], F32, kind="ExternalInput")
    gamma = nc.dram_tensor("gamma", [HIDDEN], F32, kind="ExternalInput")
    beta = nc.dram_tensor("beta", [HIDDEN], F32, kind="ExternalInput")
    w_qkv = nc.dram_tensor("w_qkv", [128, KO, 3 * QK], BF16, kind="ExternalInput")
    w_p = nc.dram_tensor("w_p", [128, KO, PCOLS], BF16, kind="ExternalInput")
    w_oo = nc.dram_tensor("w_oo", [128, 4, HIDDEN], BF16, kind="ExternalInput")
    w_op = nc.dram_tensor("w_op", [128, 16, HIDDEN], BF16, kind="ExternalInput")
    bvec = nc.dram_tensor("bvec", [HIDDEN], F32, kind="ExternalInput")
    cos_t = nc.dram_tensor("cos_t", [T, 32], F32, kind="ExternalInput")
    sin_t = nc.dram_tensor("sin_t", [T, 32], F32, kind="ExternalInput")
    out = nc.dram_tensor("out", [T, HIDDEN], F32, kind="ExternalOutput")
    if debug:
        pt_dbg = nc.dram_tensor("pt_dbg", [16, 128, T], BF16, kind="ExternalOutput")
        qt_dbg = nc.dram_tensor("qt_dbg", [128, 4, T], BF16, kind="ExternalOutput")
        kt_dbg = nc.dram_tensor("kt_dbg", [128, 4, T], BF16, kind="ExternalOutput")
        va_dbg = nc.dram_tensor("va_dbg", [128, JC, HG, 65], BF16, kind="ExternalOutput")
        ot_dbg = nc.dram_tensor("ot_dbg", [128, 4, T], BF16, kind="ExternalOutput")

    def bcast_ap(vec_ap, parts=128):
        return bass.AP(tensor=vec_ap.tensor, offset=vec_ap.offset,
                       ap=[[0, parts]] + list(vec_ap.ap))

    xn_dram = nc.dram_tensor("xn_dram", [T, HIDDEN], BF16)
    q_dram = nc.dram_tensor("q_dram", [T, QK], BF16)
    k_dram = nc.dram_tensor("k_dram", [T, QK], BF16)
    pt_dram = nc.dram_tensor("pt_dram", [16, 128, T], BF16)

    with tile.TileContext(nc) as tc:

        # ---- long-lived tensors ------------------------------------------
        persist_cm = tc.tile_pool(name="persist", bufs=1)
        persist = persist_cm.__enter__()
        qT = persist.tile([128, 4, T], BF16)           # 16 KB/part
        kT = persist.tile([128, 4, T], BF16)           # 16
        v_aug = persist.tile([128, JC, HG, 65], BF16)  # 16.3
        oT = persist.tile([128, 4, T], BF16)           # 16
        nc.vector.memset(v_aug[:, :, :, 64], 1.0)

        p3w_cm = tc.tile_pool(name="p3w", bufs=1)
        p3w = p3w_cm.__enter__()
        w_op_sb = p3w.tile([128, 16, HIDDEN], BF16)    # 32 KB/part
        w_oo_sb = p3w.tile([128, 4, HIDDEN], BF16)     # 8
        bvec_sb = p3w.tile([128, HIDDEN], F32)         # 4

        xnp_cm = tc.tile_pool(name="xnp", bufs=1)
        xnp = xnp_cm.__enter__()
        xnT = xnp.tile([128, KO, T], BF16)             # 32 KB/part

        # ---- phase 1: baseline stage A+B1 (proven-correct pattern) -------
        with tc.tile_pool(name="ln", bufs=3) as ln_pool, \
             tc.tile_pool(name="b1w", bufs=1) as b1w, \
             tc.tile_pool(name="b1t", bufs=3) as b1t, \
             tc.tile_pool(name="b1_ps", bufs=2, space="PSUM") as b1_ps:
            gamma_sb = b1w.tile([128, HIDDEN], F32)
            beta_sb = b1w.tile([128, HIDDEN], F32)
            eps_sb = b1w.tile([128, 1], F32)
            nc.gpsimd.dma_start(out=gamma_sb[:], in_=bcast_ap(gamma.ap()))
            nc.gpsimd.dma_start(out=beta_sb[:], in_=bcast_ap(beta.ap()))
            nc.vector.memset(eps_sb[:], LN_EPS)
            cos_sb = b1w.tile([128, TT, 32], F32)
            sin_sb = b1w.tile([128, TT, 32], F32)
            nc.sync.dma_start(cos_sb[:], cos_t.ap().rearrange("(t p) f -> p t f", p=128))
            nc.sync.dma_start(sin_sb[:], sin_t.ap().rearrange("(t p) f -> p t f", p=128))
            wt = b1w.tile([128, KO, 3 * QK], BF16)
            nc.sync.dma_start(wt[:], w_qkv[:])

            def rotary_evict(ps, tt, dram):
                # ps: [128 tok, 512] psum view [128, h, 2, 32]
                pr = ps[:].rearrange("p (h two f) -> p h two f", h=HG, two=2)
                cosb = cos_sb[:, tt, None, :].to_broadcast((128, HG, 32))
                sinb = sin_sb[:, tt, None, :].to_broadcast((128, HG, 32))
                rot = b1t.tile([128, HG, 2, 32], BF16, tag="rot")
                ta = b1t.tile([128, HG, 32], F32, tag="ta")
                tb = b1t.tile([128, HG, 32], F32, tag="tb")
                nc.vector.tensor_mul(ta[:], pr[:, :, 1, :], sinb)
                nc.vector.tensor_mul(tb[:], pr[:, :, 0, :], cosb)
                nc.vector.tensor_sub(rot[:, :, 0, :], tb[:], ta[:])
                nc.vector.tensor_mul(ta[:], pr[:, :, 0, :], sinb)
                nc.vector.tensor_mul(tb[:], pr[:, :, 1, :], cosb)
                nc.vector.tensor_add(rot[:, :, 1, :], tb[:], ta[:])
                nc.scalar.dma_start(
                    dram[tt * 128:(tt + 1) * 128, :],
                    rot[:].rearrange("p h two f -> p (h two f)"))

            for half in range(4):
                tts = range(half * (TT // 4), (half + 1) * (TT // 4))
                hsl = slice(half * (T // 4), (half + 1) * (T // 4))
                for tt in tts:
                    rsl = slice(tt * 128, (tt + 1) * 128)
                    xt = ln_pool.tile([128, HIDDEN], F32, tag="xt")
                    nc.gpsimd.dma_start(out=xt[:], in_=x.ap()[rsl, :])
                    stats = ln_pool.tile([128, 2, 6], F32, tag="st")
                    xr = xt[:].rearrange("p (s d) -> p s d", s=2)
                    for i in range(2):
                        nc.vector.bn_stats(out=stats[:, i, :], in_=xr[:, i, :])
                    mv = ln_pool.tile([128, 2], F32, tag="mv")
                    nc.vector.bn_aggr(out=mv[:], in_=stats[:])
                    std = ln_pool.tile([128, 1], F32, tag="sd")
                    nc.scalar.activation(out=std[:], in_=mv[:, 1:2], func=AF.Sqrt,
                                         bias=eps_sb[:])
                    rstd = ln_pool.tile([128, 1], F32, tag="rs")
                    nc.vector.reciprocal(out=rstd[:], in_=std[:])
                    xnb = ln_pool.tile([128, HIDDEN], BF16, tag="xnb")
                    if trivial_ln:
                        nc.vector.tensor_scalar(out=xnb[:], in0=xt[:],
                                                scalar1=mv[:, 0:1],
                                                scalar2=rstd[:],
                                                op0=ALU.subtract, op1=ALU.mult)
                    else:
                        nc.vector.tensor_scalar(out=xt[:], in0=xt[:],
                                                scalar1=mv[:, 0:1],
                                                scalar2=rstd[:],
                                                op0=ALU.subtract, op1=ALU.mult)
                        nc.gpsimd.tensor_mul(xt[:], xt[:], gamma_sb[:])
                        nc.vector.tensor_add(xnb[:], xt[:], beta_sb[:])
                    nc.sync.dma_start(xn_dram[rsl, :], xnb[:])
                for ks in range(KO):
                    nc.sync.dma_start_transpose(
                        xnT[:, ks, hsl],
                        xn_dram.ap()[hsl, ks * 128:(ks + 1) * 128])
                for tt in tts:
                    tsl = slice(tt * 128, (tt + 1) * 128)
                    psq = b1_ps.tile([128, QK], F32, tag="mq")
                    psk = b1_ps.tile([128, QK], F32, tag="mk")
                    psv = b1_ps.tile([128, QK], F32, tag="mv")
                    for ks in range(KO):
                        st, sp = (ks == 0), (ks == KO - 1)
                        nc.tensor.matmul(psq[:], xnT[:, ks, tsl], wt[:, ks, 0:QK],
                                         start=st, stop=sp)
                        nc.tensor.matmul(psk[:], xnT[:, ks, tsl], wt[:, ks, QK:2 * QK],
                                         start=st, stop=sp)
                        nc.tensor.matmul(psv[:], xnT[:, ks, tsl], wt[:, ks, 2 * QK:],
                                         start=st, stop=sp)
                    rotary_evict(psq, tt, q_dram)
                    rotary_evict(psk, tt, k_dram)
                    pv = psv[:].rearrange("p (h d) -> p h d", h=HG)
                    nc.vector.tensor_copy(out=v_aug[:, tt, :, 0:64], in_=pv)
                for hc in range(4):
                    nc.scalar.dma_start_transpose(
                        qT[:, hc, hsl], q_dram.ap()[hsl, hc * 128:(hc + 1) * 128])
                    nc.scalar.dma_start_transpose(
                        kT[:, hc, hsl], k_dram.ap()[hsl, hc * 128:(hc + 1) * 128])

        # phase-3 weight loads stream during phase 2
        nc.sync.dma_start(w_op_sb[:], w_op[:])
        nc.sync.dma_start(w_oo_sb[:], w_oo[:])
        nc.gpsimd.dma_start(out=bvec_sb[:], in_=bcast_ap(bvec.ap()))

        # ---- phase 2: p projection + GELU -> pt_dram ---------------------
        with tc.tile_pool(name="p2w", bufs=3) as p2w, \
             tc.tile_pool(name="p2g", bufs=3) as p2g, \
             tc.tile_pool(name="p2ps", bufs=2, space="PSUM") as p2ps:
            for pc in range(16):
                wpt = p2w.tile([128, KO, 128], BF16, tag="wp")
                nc.gpsimd.dma_start(out=wpt[:], in_=w_p.ap()[:, :, pc * 128:(pc + 1) * 128])
                for ic in range(NIC):
                    isl = slice(ic * 512, (ic + 1) * 512)
                    pp = p2ps.tile([128, 512], F32, tag="pp")
                    for ks in range(KO):
                        nc.tensor.matmul(pp[:], wpt[:, ks, :], xnT[:, ks, isl],
                                         start=(ks == 0), stop=(ks == KO - 1))
                    pt_sb = p2g.tile([128, 512], BF16, tag="pt")
                    nc.scalar.activation(pt_sb[:], pp[:], AF.Gelu)
                    nc.sync.dma_start(pt_dram.ap()[pc, :, isl], pt_sb[:])

        xnp_cm.__exit__(None, None, None)

        # ---- phase 3: attention with out_proj interleave -----------------
        with tc.tile_pool(name="pt3", bufs=1) as pt3, \
             tc.tile_pool(name="e3", bufs=3) as e3, \
             tc.tile_pool(name="n3", bufs=2) as n3, \
             tc.tile_pool(name="d3", bufs=2) as d3, \
             tc.tile_pool(name="s_ps", bufs=2, space="PSUM") as s_ps_pool, \
             tc.tile_pool(name="po_ps", bufs=1, space="PSUM") as po_ps_pool, \
             tc.tile_pool(name="o_ps", bufs=2, space="PSUM") as o_ps_pool:
            pt_a = pt3.tile([128, 16, 512], BF16)
            pt_b = pt3.tile([128, 16, 512], BF16)
            pt_tiles = [pt_a, pt_b]

            def ptload_closures(j):
                ptt = pt_tiles[j % 2]
                jsl = slice(j * 512, (j + 1) * 512)
                cls = []
                for pc in range(16):
                    def f(pc=pc, ptt=ptt, jsl=jsl):
                        nc.sync.dma_start(ptt[:, pc, :], pt_dram.ap()[pc, :, jsl])
                    cls.append(f)
                return cls

            def outproj_closures(j):
                ptt = pt_tiles[j % 2]
                state = {}
                cls = []
                for isub in range(4):
                    tok0 = j * 512 + isub * 128
                    ssl = slice(isub * 128, (isub + 1) * 128)
                    for oc in range(2):
                        osl = slice(oc * 512, (oc + 1) * 512)
                        for pc in range(16):
                            def f(pc=pc, ssl=ssl, osl=osl, ptt=ptt):
                                if pc == 0:
                                    state['po2'] = o_ps_pool.tile(
                                        [128, 512], F32, tag="po2",
                                        name="po2")
                                nc.tensor.matmul(state['po2'][:], ptt[:, pc, ssl],
                                                 w_op_sb[:, pc, osl],
                                                 start=(pc == 0), stop=False)
                            cls.append(f)
                        for ks in range(4):
                            def f(ks=ks, tok0=tok0, osl=osl):
                                nc.tensor.matmul(state['po2'][:],
                                                 oT[:, ks, tok0:tok0 + 128],
                                                 w_oo_sb[:, ks, osl],
                                                 start=False, stop=(ks == 3))
                            cls.append(f)
                        def f(oc=oc, osl=osl, tok0=tok0):
                            if oc == 0:
                                state['fin'] = d3.tile([128, HIDDEN], F32,
                                                       tag="fin", name="fin")
                            nc.vector.tensor_add(state['fin'][:, osl],
                                                 state['po2'][:], bvec_sb[:, osl])
                            if oc == 1:
                                nc.sync.dma_start(out.ap()[tok0:tok0 + 128, :],
                                                  state['fin'][:])
                        cls.append(f)
                return cls

            for ic in range(NIC):
                isl = slice(ic * 512, (ic + 1) * 512)
                fillers = ptload_closures(ic)
                if ic > 0:
                    fillers += outproj_closures(ic - 1)
                fi = 0
                for pr in range(4):
                    po_pair = po_ps_pool.tile([65, 1024], F32, tag="po")
                    prev_e = None
                    for jc in range(JC):
                        jsl = slice(jc * 128, (jc + 1) * 128)
                        s_pair = s_ps_pool.tile([128, 1024], F32, tag="s")
                        nc.tensor.matmul(s_pair[:, 0:512], kT[0:64, pr, jsl],
                                         qT[0:64, pr, isl], start=True, stop=True)
                        nc.tensor.matmul(s_pair[:, 512:1024], kT[64:128, pr, jsl],
                                         qT[64:128, pr, isl], start=True, stop=True)
                        e_pair = e3.tile([128, 1024], BF16, tag="e")
                        nc.scalar.activation(e_pair[:], s_pair[:], AF.Exp,
                                             scale=0.125)
                        for _ in range(3):
                            if fi < len(fillers):
                                fillers[fi]()
                                fi += 1
                        if prev_e is not None:
                            jm = jc - 1
                            nc.tensor.matmul(po_pair[:, 0:512],
                                             v_aug[:, jm, 2 * pr, :],
                                             prev_e[:, 0:512],
                                             start=(jm == 0), stop=False)
                            nc.tensor.matmul(po_pair[:, 512:1024],
                                             v_aug[:, jm, 2 * pr + 1, :],
                                             prev_e[:, 512:1024],
                                             start=(jm == 0), stop=False)
                        prev_e = e_pair
                    nc.tensor.matmul(po_pair[:, 0:512], v_aug[:, 15, 2 * pr, :],
                                     prev_e[:, 0:512], start=False, stop=True)
                    nc.tensor.matmul(po_pair[:, 512:1024],
                                     v_aug[:, 15, 2 * pr + 1, :],
                                     prev_e[:, 512:1024], start=False, stop=True)
                    po_sb = n3.tile([65, 1024], F32, tag="posb")
                    nc.vector.tensor_copy(out=po_sb[:], in_=po_pair[:])
                    rz = n3.tile([1, 1024], F32, tag="rz")
                    nc.vector.tensor_copy(out=rz[:], in_=po_sb[64:65, :])
                    rzb = n3.tile([64, 1024], F32, tag="rzb")
                    nc.gpsimd.partition_broadcast(rzb[:], rz[:])
                    nc.vector.reciprocal_approx_fast(rzb[:], rzb[:])
                    nc.vector.tensor_mul(oT[0:64, pr, isl], po_sb[0:64, 0:512],
                                         rzb[:, 0:512])
                    nc.vector.tensor_mul(oT[64:128, pr, isl],
                                         po_sb[0:64, 512:1024],
                                         rzb[:, 512:1024])
                while fi < len(fillers):
                    fillers[fi]()
                    fi += 1

            # ---- phase 4: out_proj tail for the last window --------------
            for f in outproj_closures(NIC - 1):
                f()

        if debug:
            nc.sync.dma_start(ot_dbg.ap(), oT[:])
            nc.sync.dma_start(qt_dbg.ap(), qT[:])
            nc.sync.dma_start(kt_dbg.ap(), kT[:])
            nc.sync.dma_start(va_dbg.ap(), v_aug[:])
            nc.sync.dma_start(pt_dbg.ap(), pt_dram.ap())
        p3w_cm.__exit__(None, None, None)
        persist_cm.__exit__(None, None, None)

    nc.compile()
    return nc


_NC_CACHE = {}


def _get_nc(trivial_ln):
    if trivial_ln not in _NC_CACHE:
        _NC_CACHE[trivial_ln] = _build_nc(trivial_ln)
    return _NC_CACHE[trivial_ln]


def _host_tables():
    inv_freq = 1.0 / (10000.0 ** (np.arange(0, HEAD_DIM, 2, dtype=np.float32)
                                  / HEAD_DIM))
    ang = np.arange(T, dtype=np.float32)[:, None] * inv_freq[None, :]
    return np.cos(ang).astype(np.float32), np.sin(ang).astype(np.float32)


def _shard_weights(w_in, w_out, b_out, ln_gamma, ln_beta, x):
    cos_np, sin_np = _host_tables()
    bf = ml_dtypes.bfloat16

    def fold(a, ko):
        # [ko*128, c] -> [128, ko, c] with [p, k, c] = a[k*128 + p, c]
        return np.ascontiguousarray(
            a.reshape(ko, 128, a.shape[1]).transpose(1, 0, 2))

    in_maps = []
    for c in range(N_CORES):
        b, g = c // 2, c % 2
        sl = slice(g * QK, (g + 1) * QK)
        w_qkv = np.concatenate(
            [w_in[:, 0 * HIDDEN:][:, sl], w_in[:, 1 * HIDDEN:][:, sl],
             w_in[:, 2 * HIDDEN:][:, sl]], axis=1)
        w_p = w_in[:, 3 * HIDDEN + g * PCOLS:3 * HIDDEN + (g + 1) * PCOLS]
        w_oo = w_out[g * QK:(g + 1) * QK, :]
        w_op = w_out[HIDDEN + g * PCOLS:HIDDEN + (g + 1) * PCOLS, :]
        in_maps.append({
            "x": np.ascontiguousarray(x[b]).astype(np.float32),
            "gamma": ln_gamma.astype(np.float32),
            "beta": ln_beta.astype(np.float32),
            "w_qkv": fold(w_qkv, KO).astype(bf),
            "w_p": fold(w_p, KO).astype(bf),
            "w_oo": fold(w_oo, 4).astype(bf),
            "w_op": fold(w_op, 16).astype(bf),
            "bvec": (b_out if g == 0 else np.zeros_like(b_out)).astype(np.float32),
            "cos_t": cos_np,
            "sin_t": sin_np,
        })
    return in_maps


def kernel(x, ln_gamma, ln_beta, w_in, w_out, b_out, _trace=False, _tmpdir=None):
    x = np.asarray(x, dtype=np.float32)
    ln_gamma = np.asarray(ln_gamma, dtype=np.float32)
    ln_beta = np.asarray(ln_beta, dtype=np.float32)
    w_in = np.asarray(w_in, dtype=np.float32)
    w_out = np.asarray(w_out, dtype=np.float32)
    b_out = np.asarray(b_out, dtype=np.float32)

    trivial_ln = bool(np.allclose(ln_gamma, 1.0) and np.allclose(ln_beta, 0.0))
    nc = _get_nc(trivial_ln)
    in_maps = _shard_weights(w_in, w_out, b_out, ln_gamma, ln_beta, x)
    kwargs = {}
    if _trace:
        kwargs = {"trace": True, "tmpdir": _tmpdir}
    res = None
    last_err = None
    for _attempt in range(3):
        try:
            res = run_bass_kernel_spmd(nc, in_maps,
                                       core_ids=list(range(N_CORES)), **kwargs)
            break
        except Exception as e:  # transient device flakes (NRT_EXEC_UNIT_...)
            last_err = e
    if res is None:
        raise last_err
    outs = [res.results[c]["out"] for c in range(N_CORES)]
    full = np.stack([outs[2 * b] + outs[2 * b + 1] for b in range(B)], axis=0)
    kernel._last_exec_time_ns = res.exec_time_ns
    return full.astype(np.float32)


# revision 30
# speedup vs baseline: 1.0069x; 1.0069x over previous
"""Trainium2 Bass kernel for nn_Block_90726889161490 (sparse_attention).

Reference computation (B=4, T=2048, HIDDEN=1024, 16 heads x 64):
    LayerNorm -> fused qkvp projection (7*HIDDEN cols) -> identity seq
    "compression" (scale 1.0) -> rotary(q, k) -> full softmax attention ->
    GELU side branch on p -> concat([o, p]) @ w_out + b_out.

Sharding: 8 cores = 4 batches x 2 head-groups (tensor parallel over heads
for q/k/v/attention, column split of in_proj, row split of out_proj).
Each core computes a partial [T, HIDDEN] output; host sums the two
head-group partials per batch (the all-reduce after out_proj).

Per-core schedule (three ACT-table-clean phases):
  P1: LN (bn_stats+Sqrt) -> xn_dram[ks][T,128] -> contiguous XBAR
      transposes -> xnT; qkv psum chains; rotary on SBUF bf16 (4 DVE ops)
      -> q_dram/k_dram[hc][T,128] -> qT/kT transposes; v -> v_aug
      (65th col ones for the softmax denominator).
  P2: p projection in [pcol, tok] layout (w_p stationary, reused across
      token chunks) + exact GELU -> pt_dram.
  P3: per 512-token i-window: per head-pair, per j-chunk: row-tiled QK^T
      (2 heads concurrent on PE row groups 0-1/2-3), one fused exp over
      both heads' scores [128,1024], A^T V psum chains with the ones
      column giving denominators; out_proj MMs of the previous window
      interleaved into the PE stream to hide ACT exp time; normalize
      via reciprocal + gpsimd partition broadcast -> oT.
  P4: out_proj tail for the last window.
"""

import os
import sys

for _p in ("/opt/trn_rl_repo", "/root/.axon_site/_ro/trn_rl_repo"):
    if os.path.isdir(_p) and _p not in sys.path:
        sys.path.insert(0, _p)

import numpy as np
import ml_dtypes

import concourse.bass as bass
import concourse.mybir as mybir
import concourse.tile as tile
from concourse import bacc
from concourse.bass_utils import run_bass_kernel_spmd

F32 = mybir.dt.float32
BF16 = mybir.dt.bfloat16
AF = mybir.ActivationFunctionType
ALU = mybir.AluOpType

N_CORES = 8
B, T, HIDDEN = 4, 2048, 1024
HEADS, HEAD_DIM = 16, 64
HG = HEADS // 2          # heads per core = 8
QK = HG * HEAD_DIM       # q/k/v col-slice per core = 512
PCOLS = 4 * HIDDEN // 2  # p col-slice per core = 2048
KO = HIDDEN // 128       # 8 contraction subtiles for d=1024
TT = T // 128            # 16 token tiles
NIC = 4                  # attention i-windows of 512 tokens
JC = T // 128            # 16 attention j-chunks
LN_EPS = 1e-5


def _build_nc(trivial_ln, debug=False):
    nc = bacc.Bacc("TRN2", target_bir_lowering=False, debug=False)

    x = nc.dram_tensor("x", [T, HIDDEN], F32, kind="ExternalInput")
    gamma = nc.dram_tensor("gamma", [HIDDEN], F32, kind="ExternalInput")
    beta = nc.dram_tensor("beta", [HIDDEN], F32, kind="ExternalInput")
    w_qkv = nc.dram_tensor("w_qkv", [128, KO, 3 * QK], BF16, kind="ExternalInput")
    w_p = nc.dram_tensor("w_p", [128, KO, PCOLS], BF16, kind="ExternalInput")
    w_oo = nc.dram_tensor("w_oo", [128, 4, HIDDEN], BF16, kind="ExternalInput")
    w_op = nc.dram_tensor("w_op", [128, 16, HIDDEN], BF16, kind="ExternalInput")
    bvec = nc.dram_tensor("bvec", [HIDDEN], F32, kind="ExternalInput")
    cos_t = nc.dram_tensor("cos_t", [T, 32], F32, kind="ExternalInput")
    sin_t = nc.dram_tensor("sin_t", [T, 32], F32, kind="ExternalInput")
    out = nc.dram_tensor("out", [T, HIDDEN], F32, kind="ExternalOutput")
    if debug:
        pt_dbg = nc.dram_tensor("pt_dbg", [16, 128, T], BF16, kind="ExternalOutput")
        qt_dbg = nc.dram_tensor("qt_dbg", [128, 4, T], BF16, kind="ExternalOutput")
        kt_dbg = nc.dram_tensor("kt_dbg", [128, 4, T], BF16, kind="ExternalOutput")
        va_dbg = nc.dram_tensor("va_dbg", [128, JC, HG, 65], BF16, kind="ExternalOutput")
        ot_dbg = nc.dram_tensor("ot_dbg", [128, 4, T], BF16, kind="ExternalOutput")

    def bcast_ap(vec_ap, parts=128):
        return bass.AP(tensor=vec_ap.tensor, offset=vec_ap.offset,
                       ap=[[0, parts]] + list(vec_ap.ap))

    xn_dram = nc.dram_tensor("xn_dram", [T, HIDDEN], BF16)
    q_dram = nc.dram_tensor("q_dram", [T, QK], BF16)
    k_dram = nc.dram_tensor("k_dram", [T, QK], BF16)
    pt_dram = nc.dram_tensor("pt_dram", [16, 128, T], BF16)

    with tile.TileContext(nc) as tc:

        # ---- long-lived tensors ------------------------------------------
        persist_cm = tc.tile_pool(name="persist", bufs=1)
        persist = persist_cm.__enter__()
        qT = persist.tile([128, 4, T], BF16)           # 16 KB/part
        kT = persist.tile([128, 4, T], BF16)           # 16
        v_aug = persist.tile([128, JC, HG, 65], BF16)  # 16.3
        oT = persist.tile([128, 4, T], BF16)           # 16
        nc.vector.memset(v_aug[:, :, :, 64], 1.0)

        p3w_cm = tc.tile_pool(name="p3w", bufs=1)
        p3w = p3w_cm.__enter__()
        w_op_sb = p3w.tile([128, 16, HIDDEN], BF16)    # 32 KB/part
        w_oo_sb = p3w.tile([128, 4, HIDDEN], BF16)     # 8
        bvec_sb = p3w.tile([128, HIDDEN], F32)         # 4

        xnp_cm = tc.tile_pool(name="xnp", bufs=1)
        xnp = xnp_cm.__enter__()
        xnT = xnp.tile([128, KO, T], BF16)             # 32 KB/part

        # ---- phase 1: baseline stage A+B1 (proven-correct pattern) -------
        with tc.tile_pool(name="ln", bufs=3) as ln_pool, \
             tc.tile_pool(name="b1w", bufs=1) as b1w, \
             tc.tile_pool(name="b1t", bufs=3) as b1t, \
             tc.tile_pool(name="b1_ps", bufs=2, space="PSUM") as b1_ps:
            gamma_sb = b1w.tile([128, HIDDEN], F32)
            beta_sb = b1w.tile([128, HIDDEN], F32)
            eps_sb = b1w.tile([128, 1], F32)
            nc.gpsimd.dma_start(out=gamma_sb[:], in_=bcast_ap(gamma.ap()))
            nc.gpsimd.dma_start(out=beta_sb[:], in_=bcast_ap(beta.ap()))
            nc.vector.memset(eps_sb[:], LN_EPS)
            cos_sb = b1w.tile([128, TT, 32], F32)
            sin_sb = b1w.tile([128, TT, 32], F32)
            nc.sync.dma_start(cos_sb[:], cos_t.ap().rearrange("(t p) f -> p t f", p=128))
            nc.sync.dma_start(sin_sb[:], sin_t.ap().rearrange("(t p) f -> p t f", p=128))
            wt = b1w.tile([128, KO, 3 * QK], BF16)
            nc.sync.dma_start(wt[:], w_qkv[:])

            def rotary_evict(ps, tt, dram):
                # ps: [128 tok, 512] psum view [128, h, 2, 32]
                pr = ps[:].rearrange("p (h two f) -> p h two f", h=HG, two=2)
                cosb = cos_sb[:, tt, None, :].to_broadcast((128, HG, 32))
                sinb = sin_sb[:, tt, None, :].to_broadcast((128, HG, 32))
                rot = b1t.tile([128, HG, 2, 32], BF16, tag="rot")
                ta = b1t.tile([128, HG, 32], F32, tag="ta")
                tb = b1t.tile([128, HG, 32], F32, tag="tb")
                nc.vector.tensor_mul(ta[:], pr[:, :, 1, :], sinb)
                nc.vector.tensor_mul(tb[:], pr[:, :, 0, :], cosb)
                nc.vector.tensor_sub(rot[:, :, 0, :], tb[:], ta[:])
                nc.vector.tensor_mul(ta[:], pr[:, :, 0, :], sinb)
                nc.vector.tensor_mul(tb[:], pr[:, :, 1, :], cosb)
                nc.vector.tensor_add(rot[:, :, 1, :], tb[:], ta[:])
                nc.scalar.dma_start(
                    dram[tt * 128:(tt + 1) * 128, :],
                    rot[:].rearrange("p h two f -> p (h two f)"))

            for half in range(4):
                tts = range(half * (TT // 4), (half + 1) * (TT // 4))
                hsl = slice(half * (T // 4), (half + 1) * (T // 4))
                for tt in tts:
                    rsl = slice(tt * 128, (tt + 1) * 128)
                    xt = ln_pool.tile([128, HIDDEN], F32, tag="xt")
                    nc.gpsimd.dma_start(out=xt[:], in_=x.ap()[rsl, :])
                    stats = ln_pool.tile([128, 2, 6], F32, tag="st")
                    xr = xt[:].rearrange("p (s d) -> p s d", s=2)
                    for i in range(2):
                        nc.vector.bn_stats(out=stats[:, i, :], in_=xr[:, i, :])
                    mv = ln_pool.tile([128, 2], F32, tag="mv")
                    nc.vector.bn_aggr(out=mv[:], in_=stats[:])
                    std = ln_pool.tile([128, 1], F32, tag="sd")
                    nc.scalar.activation(out=std[:], in_=mv[:, 1:2], func=AF.Sqrt,
                                         bias=eps_sb[:])
                    rstd = ln_pool.tile([128, 1], F32, tag="rs")
                    nc.vector.reciprocal(out=rstd[:], in_=std[:])
                    xnb = ln_pool.tile([128, HIDDEN], BF16, tag="xnb")
                    if trivial_ln:
                        nc.vector.tensor_scalar(out=xnb[:], in0=xt[:],
                                                scalar1=mv[:, 0:1],
                                                scalar2=rstd[:],
                                                op0=ALU.subtract, op1=ALU.mult)
                    else:
                        nc.vector.tensor_scalar(out=xt[:], in0=xt[:],
                                                scalar1=mv[:, 0:1],
                                                scalar2=rstd[:],
                                                op0=ALU.subtract, op1=ALU.mult)
                        nc.gpsimd.tensor_mul(xt[:], xt[:], gamma_sb[:])
                        nc.vector.tensor_add(xnb[:], xt[:], beta_sb[:])
                    nc.sync.dma_start(xn_dram[rsl, :], xnb[:])
                for ks in range(KO):
                    nc.sync.dma_start_transpose(
                        xnT[:, ks, hsl],
                        xn_dram.ap()[hsl, ks * 128:(ks + 1) * 128])
                for tt in tts:
                    tsl = slice(tt * 128, (tt + 1) * 128)
                    psq = b1_ps.tile([128, QK], F32, tag="mq", bufs=3)
                    psk = b1_ps.tile([128, QK], F32, tag="mk", bufs=3)
                    psv = b1_ps.tile([128, QK], F32, tag="mv")
                    for ks in range(KO):
                        st, sp = (ks == 0), (ks == KO - 1)
                        nc.tensor.matmul(psq[:], xnT[:, ks, tsl], wt[:, ks, 0:QK],
                                         start=st, stop=sp)
                        nc.tensor.matmul(psk[:], xnT[:, ks, tsl], wt[:, ks, QK:2 * QK],
                                         start=st, stop=sp)
                        nc.tensor.matmul(psv[:], xnT[:, ks, tsl], wt[:, ks, 2 * QK:],
                                         start=st, stop=sp)
                    rotary_evict(psq, tt, q_dram)
                    rotary_evict(psk, tt, k_dram)
                    pv = psv[:].rearrange("p (h d) -> p h d", h=HG)
                    nc.vector.tensor_copy(out=v_aug[:, tt, :, 0:64], in_=pv)
                for hc in range(4):
                    nc.scalar.dma_start_transpose(
                        qT[:, hc, hsl], q_dram.ap()[hsl, hc * 128:(hc + 1) * 128])
                    nc.scalar.dma_start_transpose(
                        kT[:, hc, hsl], k_dram.ap()[hsl, hc * 128:(hc + 1) * 128])

        # phase-3 weight loads stream during phase 2
        nc.sync.dma_start(w_op_sb[:], w_op[:])
        nc.sync.dma_start(w_oo_sb[:], w_oo[:])
        nc.gpsimd.dma_start(out=bvec_sb[:], in_=bcast_ap(bvec.ap()))

        # ---- phase 2: p projection + GELU -> pt_dram ---------------------
        with tc.tile_pool(name="p2w", bufs=3) as p2w, \
             tc.tile_pool(name="p2g", bufs=3) as p2g, \
             tc.tile_pool(name="p2ps", bufs=2, space="PSUM") as p2ps:
            for pc in range(16):
                wpt = p2w.tile([128, KO, 128], BF16, tag="wp")
                nc.gpsimd.dma_start(out=wpt[:], in_=w_p.ap()[:, :, pc * 128:(pc + 1) * 128])
                for ic in range(NIC):
                    isl = slice(ic * 512, (ic + 1) * 512)
                    pp = p2ps.tile([128, 512], F32, tag="pp")
                    for ks in range(KO):
                        nc.tensor.matmul(pp[:], wpt[:, ks, :], xnT[:, ks, isl],
                                         start=(ks == 0), stop=(ks == KO - 1))
                    pt_sb = p2g.tile([128, 512], BF16, tag="pt")
                    nc.scalar.activation(pt_sb[:], pp[:], AF.Gelu)
                    nc.sync.dma_start(pt_dram.ap()[pc, :, isl], pt_sb[:])

        xnp_cm.__exit__(None, None, None)

        # ---- phase 3: attention with out_proj interleave -----------------
        with tc.tile_pool(name="pt3", bufs=1) as pt3, \
             tc.tile_pool(name="e3", bufs=3) as e3, \
             tc.tile_pool(name="n3", bufs=2) as n3, \
             tc.tile_pool(name="d3", bufs=2) as d3, \
             tc.tile_pool(name="s_ps", bufs=2, space="PSUM") as s_ps_pool, \
             tc.tile_pool(name="po_ps", bufs=1, space="PSUM") as po_ps_pool, \
             tc.tile_pool(name="o_ps", bufs=2, space="PSUM") as o_ps_pool:
            pt_a = pt3.tile([128, 16, 512], BF16)
            pt_b = pt3.tile([128, 16, 512], BF16)
            pt_tiles = [pt_a, pt_b]

            def ptload_closures(j):
                ptt = pt_tiles[j % 2]
                jsl = slice(j * 512, (j + 1) * 512)
                cls = []
                for pc in range(16):
                    def f(pc=pc, ptt=ptt, jsl=jsl):
                        nc.sync.dma_start(ptt[:, pc, :], pt_dram.ap()[pc, :, jsl])
                    cls.append(f)
                return cls

            def outproj_closures(j):
                ptt = pt_tiles[j % 2]
                state = {}
                cls = []
                for isub in range(4):
                    tok0 = j * 512 + isub * 128
                    ssl = slice(isub * 128, (isub + 1) * 128)
                    for oc in range(2):
                        osl = slice(oc * 512, (oc + 1) * 512)
                        for pc in range(16):
                            def f(pc=pc, ssl=ssl, osl=osl, ptt=ptt):
                                if pc == 0:
                                    state['po2'] = o_ps_pool.tile(
                                        [128, 512], F32, tag="po2",
                                        name="po2")
                                nc.tensor.matmul(state['po2'][:], ptt[:, pc, ssl],
                                                 w_op_sb[:, pc, osl],
                                                 start=(pc == 0), stop=False)
                            cls.append(f)
                        for ks in range(4):
                            def f(ks=ks, tok0=tok0, osl=osl):
                                nc.tensor.matmul(state['po2'][:],
                                                 oT[:, ks, tok0:tok0 + 128],
                                                 w_oo_sb[:, ks, osl],
                                                 start=False, stop=(ks == 3))
                            cls.append(f)
                        def f(oc=oc, osl=osl, tok0=tok0):
                            if oc == 0:
                                state['fin'] = d3.tile([128, HIDDEN], F32,
                                                       tag="fin", name="fin")
                            nc.vector.tensor_add(state['fin'][:, osl],
                                                 state['po2'][:], bvec_sb[:, osl])
                            if oc == 1:
                                nc.sync.dma_start(out.ap()[tok0:tok0 + 128, :],
                                                  state['fin'][:])
                        cls.append(f)
                return cls

            for ic in range(NIC):
                isl = slice(ic * 512, (ic + 1) * 512)
                fillers = ptload_closures(ic)
                if ic > 0:
                    fillers += outproj_closures(ic - 1)
                fi = 0
                for pr in range(4):
                    po_pair = po_ps_pool.tile([65, 1024], F32, tag="po")
                    prev_e = None
                    for jc in range(JC):
                        jsl = slice(jc * 128, (jc + 1) * 128)
                        s_pair = s_ps_pool.tile([128, 1024], F32, tag="s")
                        nc.tensor.matmul(s_pair[:, 0:512], kT[0:64, pr, jsl],
                                         qT[0:64, pr, isl], start=True, stop=True)
                        nc.tensor.matmul(s_pair[:, 512:1024], kT[64:128, pr, jsl],
                                         qT[64:128, pr, isl], start=True, stop=True)
                        e_pair = e3.tile([128, 1024], BF16, tag="e")
                        nc.scalar.activation(e_pair[:], s_pair[:], AF.Exp,
                                             scale=0.125)
                        for _ in range(3):
                            if fi < len(fillers):
                                fillers[fi]()
                                fi += 1
                        if prev_e is not None:
                            jm = jc - 1
                            nc.tensor.matmul(po_pair[:, 0:512],
                                             v_aug[:, jm, 2 * pr, :],
                                             prev_e[:, 0:512],
                                             start=(jm == 0), stop=False)
                            nc.tensor.matmul(po_pair[:, 512:1024],
                                             v_aug[:, jm, 2 * pr + 1, :],
                                             prev_e[:, 512:1024],
                                             start=(jm == 0), stop=False)
                        prev_e = e_pair
                    nc.tensor.matmul(po_pair[:, 0:512], v_aug[:, 15, 2 * pr, :],
                                     prev_e[:, 0:512], start=False, stop=True)
                    nc.tensor.matmul(po_pair[:, 512:1024],
                                     v_aug[:, 15, 2 * pr + 1, :],
                                     prev_e[:, 512:1024], start=False, stop=True)
                    po_sb = n3.tile([65, 1024], F32, tag="posb")
                    nc.vector.tensor_copy(out=po_sb[:], in_=po_pair[:])
                    rz = n3.tile([1, 1024], F32, tag="rz")
                    nc.vector.tensor_copy(out=rz[:], in_=po_sb[64:65, :])
                    rzb = n3.tile([64, 1024], F32, tag="rzb")
                    nc.gpsimd.partition_broadcast(rzb[:], rz[:])
                    nc.vector.reciprocal_approx_fast(rzb[:], rzb[:])
                    nc.vector.tensor_mul(oT[0:64, pr, isl], po_sb[0:64, 0:512],
                                         rzb[:, 0:512])
                    nc.vector.tensor_mul(oT[64:128, pr, isl],
                                         po_sb[0:64, 512:1024],
                                         rzb[:, 512:1024])
                while fi < len(fillers):
                    fillers[fi]()
                    fi += 1

            # ---- phase 4: out_proj tail for the last window --------------
            for f in outproj_closures(NIC - 1):
                f()

        if debug:
            nc.sync.dma_start(ot_dbg.ap(), oT[:])
            nc.sync.dma_start(qt_dbg.ap(), qT[:])
            nc.sync.dma_start(kt_dbg.ap(), kT[:])
            nc.sync.dma_start(va_dbg.ap(), v_aug[:])
            nc.sync.dma_start(pt_dbg.ap(), pt_dram.ap())
        p3w_cm.__exit__(None, None, None)
        persist_cm.__exit__(None, None, None)

    nc.compile()
    return nc


_NC_CACHE = {}


def _get_nc(trivial_ln):
    if trivial_ln not in _NC_CACHE:
        _NC_CACHE[trivial_ln] = _build_nc(trivial_ln)
    return _NC_CACHE[trivial_ln]


def _host_tables():
    inv_freq = 1.0 / (10000.0 ** (np.arange(0, HEAD_DIM, 2, dtype=np.float32)
                                  / HEAD_DIM))
    ang = np.arange(T, dtype=np.float32)[:, None] * inv_freq[None, :]
    return np.cos(ang).astype(np.float32), np.sin(ang).astype(np.float32)


def _shard_weights(w_in, w_out, b_out, ln_gamma, ln_beta, x):
    cos_np, sin_np = _host_tables()
    bf = ml_dtypes.bfloat16

    def fold(a, ko):
        # [ko*128, c] -> [128, ko, c] with [p, k, c] = a[k*128 + p, c]
        return np.ascontiguousarray(
            a.reshape(ko, 128, a.shape[1]).transpose(1, 0, 2))

    in_maps = []
    for c in range(N_CORES):
        b, g = c // 2, c % 2
        sl = slice(g * QK, (g + 1) * QK)
        w_qkv = np.concatenate(
            [w_in[:, 0 * HIDDEN:][:, sl], w_in[:, 1 * HIDDEN:][:, sl],
             w_in[:, 2 * HIDDEN:][:, sl]], axis=1)
        w_p = w_in[:, 3 * HIDDEN + g * PCOLS:3 * HIDDEN + (g + 1) * PCOLS]
        w_oo = w_out[g * QK:(g + 1) * QK, :]
        w_op = w_out[HIDDEN + g * PCOLS:HIDDEN + (g + 1) * PCOLS, :]
        in_maps.append({
            "x": np.ascontiguousarray(x[b]).astype(np.float32),
            "gamma": ln_gamma.astype(np.float32),
            "beta": ln_beta.astype(np.float32),
            "w_qkv": fold(w_qkv, KO).astype(bf),
            "w_p": fold(w_p, KO).astype(bf),
            "w_oo": fold(w_oo, 4).astype(bf),
            "w_op": fold(w_op, 16).astype(bf),
            "bvec": (b_out if g == 0 else np.zeros_like(b_out)).astype(np.float32),
            "cos_t": cos_np,
            "sin_t": sin_np,
        })
    return in_maps


def kernel(x, ln_gamma, ln_beta, w_in, w_out, b_out, _trace=False, _tmpdir=None):
    x = np.asarray(x, dtype=np.float32)
    ln_gamma = np.asarray(ln_gamma, dtype=np.float32)
    ln_beta = np.asarray(ln_beta, dtype=np.float32)
    w_in = np.asarray(w_in, dtype=np.float32)
    w_out = np.asarray(w_out, dtype=np.float32)
    b_out = np.asarray(b_out, dtype=np.float32)

    trivial_ln = bool(np.allclose(ln_gamma, 1.0) and np.allclose(ln_beta, 0.0))
    nc = _get_nc(trivial_ln)
    in_maps = _shard_weights(w_in, w_out, b_out, ln_gamma, ln_beta, x)
    kwargs = {}
    if _trace:
        kwargs = {"trace": True, "tmpdir": _tmpdir}
    res = None
    last_err = None
    for _attempt in range(3):
        try:
            res = run_bass_kernel_spmd(nc, in_maps,
                                       core_ids=list(range(N_CORES)), **kwargs)
            break
        except Exception as e:  # transient device flakes (NRT_EXEC_UNIT_...)
            last_err = e
    if res is None:
        raise last_err
    outs = [res.results[c]["out"] for c in range(N_CORES)]
    full = np.stack([outs[2 * b] + outs[2 * b + 1] for b in range(B)], axis=0)
    kernel._last_exec_time_ns = res.exec_time_ns
    return full.astype(np.float32)


# revision 32
# speedup vs baseline: 1.0262x; 1.0191x over previous
"""Trainium2 Bass kernel for nn_Block_90726889161490 (sparse_attention).

Reference computation (B=4, T=2048, HIDDEN=1024, 16 heads x 64):
    LayerNorm -> fused qkvp projection (7*HIDDEN cols) -> identity seq
    "compression" (scale 1.0) -> rotary(q, k) -> full softmax attention ->
    GELU side branch on p -> concat([o, p]) @ w_out + b_out.

Sharding: 8 cores = 4 batches x 2 head-groups (tensor parallel over heads
for q/k/v/attention, column split of in_proj, row split of out_proj).
Each core computes a partial [T, HIDDEN] output; host sums the two
head-group partials per batch (the all-reduce after out_proj).

Per-core schedule (three ACT-table-clean phases):
  P1: LN (bn_stats+Sqrt) -> xn_dram[ks][T,128] -> contiguous XBAR
      transposes -> xnT; qkv psum chains; rotary on SBUF bf16 (4 DVE ops)
      -> q_dram/k_dram[hc][T,128] -> qT/kT transposes; v -> v_aug
      (65th col ones for the softmax denominator).
  P2: p projection in [pcol, tok] layout (w_p stationary, reused across
      token chunks) + exact GELU -> pt_dram.
  P3: per 512-token i-window: per head-pair, per j-chunk: row-tiled QK^T
      (2 heads concurrent on PE row groups 0-1/2-3), one fused exp over
      both heads' scores [128,1024], A^T V psum chains with the ones
      column giving denominators; out_proj MMs of the previous window
      interleaved into the PE stream to hide ACT exp time; normalize
      via reciprocal + gpsimd partition broadcast -> oT.
  P4: out_proj tail for the last window.
"""

import os
import sys

for _p in ("/opt/trn_rl_repo", "/root/.axon_site/_ro/trn_rl_repo"):
    if os.path.isdir(_p) and _p not in sys.path:
        sys.path.insert(0, _p)

import numpy as np
import ml_dtypes

import concourse.bass as bass
import concourse.mybir as mybir
import concourse.tile as tile
from concourse import bacc
from concourse.bass_utils import run_bass_kernel_spmd

F32 = mybir.dt.float32
BF16 = mybir.dt.bfloat16
AF = mybir.ActivationFunctionType
ALU = mybir.AluOpType

N_CORES = 8
B, T, HIDDEN = 4, 2048, 1024
HEADS, HEAD_DIM = 16, 64
HG = HEADS // 2          # heads per core = 8
QK = HG * HEAD_DIM       # q/k/v col-slice per core = 512
PCOLS = 4 * HIDDEN // 2  # p col-slice per core = 2048
KO = HIDDEN // 128       # 8 contraction subtiles for d=1024
TT = T // 128            # 16 token tiles
NIC = 4                  # attention i-windows of 512 tokens
JC = T // 128            # 16 attention j-chunks
LN_EPS = 1e-5


def _build_nc(trivial_ln, debug=False):
    nc = bacc.Bacc("TRN2", target_bir_lowering=False, debug=False)

    x = nc.dram_tensor("x", [T, HIDDEN], F32, kind="ExternalInput")
    gamma = nc.dram_tensor("gamma", [HIDDEN], F32, kind="ExternalInput")
    beta = nc.dram_tensor("beta", [HIDDEN], F32, kind="ExternalInput")
    w_qkv = nc.dram_tensor("w_qkv", [128, KO, 3 * QK], BF16, kind="ExternalInput")
    w_p = nc.dram_tensor("w_p", [128, KO, PCOLS], BF16, kind="ExternalInput")
    w_oo = nc.dram_tensor("w_oo", [128, 4, HIDDEN], BF16, kind="ExternalInput")
    w_op = nc.dram_tensor("w_op", [128, 16, HIDDEN], BF16, kind="ExternalInput")
    bvec = nc.dram_tensor("bvec", [HIDDEN], F32, kind="ExternalInput")
    cos_t = nc.dram_tensor("cos_t", [T, 32], F32, kind="ExternalInput")
    sin_t = nc.dram_tensor("sin_t", [T, 32], F32, kind="ExternalInput")
    out = nc.dram_tensor("out", [T, HIDDEN], F32, kind="ExternalOutput")
    if debug:
        pt_dbg = nc.dram_tensor("pt_dbg", [16, 128, T], BF16, kind="ExternalOutput")
        qt_dbg = nc.dram_tensor("qt_dbg", [128, 4, T], BF16, kind="ExternalOutput")
        kt_dbg = nc.dram_tensor("kt_dbg", [128, 4, T], BF16, kind="ExternalOutput")
        va_dbg = nc.dram_tensor("va_dbg", [128, JC, HG, 65], BF16, kind="ExternalOutput")
        ot_dbg = nc.dram_tensor("ot_dbg", [128, 4, T], BF16, kind="ExternalOutput")

    def bcast_ap(vec_ap, parts=128):
        return bass.AP(tensor=vec_ap.tensor, offset=vec_ap.offset,
                       ap=[[0, parts]] + list(vec_ap.ap))

    xn_dram = nc.dram_tensor("xn_dram", [T, HIDDEN], BF16)
    q_dram = nc.dram_tensor("q_dram", [T, QK], BF16)
    k_dram = nc.dram_tensor("k_dram", [T, QK], BF16)
    pt_dram = nc.dram_tensor("pt_dram", [16, 128, T], BF16)

    with tile.TileContext(nc) as tc:

        # ---- long-lived tensors ------------------------------------------
        persist_cm = tc.tile_pool(name="persist", bufs=1)
        persist = persist_cm.__enter__()
        qT = persist.tile([128, 4, T], BF16)           # 16 KB/part
        kT = persist.tile([128, 4, T], BF16)           # 16
        v_aug = persist.tile([128, JC, HG, 65], BF16)  # 16.3
        oT = persist.tile([128, 4, T], BF16)           # 16
        nc.vector.memset(v_aug[:, :, :, 64], 1.0)

        p3w_cm = tc.tile_pool(name="p3w", bufs=1)
        p3w = p3w_cm.__enter__()
        w_op_sb = p3w.tile([128, 16, HIDDEN], BF16)    # 32 KB/part
        w_oo_sb = p3w.tile([128, 4, HIDDEN], BF16)     # 8
        bvec_sb = p3w.tile([128, HIDDEN], F32)         # 4

        xnp_cm = tc.tile_pool(name="xnp", bufs=1)
        xnp = xnp_cm.__enter__()
        xnT = xnp.tile([128, KO, T], BF16)             # 32 KB/part

        # ---- phase 1: baseline stage A+B1 (proven-correct pattern) -------
        with tc.tile_pool(name="ln", bufs=3) as ln_pool, \
             tc.tile_pool(name="b1w", bufs=1) as b1w, \
             tc.tile_pool(name="b1t", bufs=3) as b1t, \
             tc.tile_pool(name="b1_ps", bufs=2, space="PSUM") as b1_ps, \
             tc.tile_pool(name="warm", bufs=1, space="PSUM") as warm_pool:
            gamma_sb = b1w.tile([128, HIDDEN], F32)
            beta_sb = b1w.tile([128, HIDDEN], F32)
            eps_sb = b1w.tile([128, 1], F32)
            nc.gpsimd.dma_start(out=gamma_sb[:], in_=bcast_ap(gamma.ap()))
            nc.gpsimd.dma_start(out=beta_sb[:], in_=bcast_ap(beta.ap()))
            nc.vector.memset(eps_sb[:], LN_EPS)
            cos_sb = b1w.tile([128, TT, 32], F32)
            sin_sb = b1w.tile([128, TT, 32], F32)
            nc.sync.dma_start(cos_sb[:], cos_t.ap().rearrange("(t p) f -> p t f", p=128))
            nc.sync.dma_start(sin_sb[:], sin_t.ap().rearrange("(t p) f -> p t f", p=128))
            wt = b1w.tile([128, KO, 3 * QK], BF16)
            nc.sync.dma_start(wt[:], w_qkv[:])
            warm_ps = warm_pool.tile([128, 128], F32)

            def warm(n):
                # tiny dead matmuls: keep the PE HAM clock-gate at 8/8
                # through phase 1's DMA/DVE-bound stretches
                for _ in range(n):
                    nc.tensor.matmul(warm_ps[:], wt[:, 0, 0:128],
                                     wt[:, 0, 0:128], start=True, stop=True)

            def rotary_evict(ps, tt, dram):
                # ps: [128 tok, 512] psum view [128, h, 2, 32]
                pr = ps[:].rearrange("p (h two f) -> p h two f", h=HG, two=2)
                cosb = cos_sb[:, tt, None, :].to_broadcast((128, HG, 32))
                sinb = sin_sb[:, tt, None, :].to_broadcast((128, HG, 32))
                rot = b1t.tile([128, HG, 2, 32], BF16, tag="rot")
                ta = b1t.tile([128, HG, 32], F32, tag="ta")
                tb = b1t.tile([128, HG, 32], F32, tag="tb")
                nc.vector.tensor_mul(ta[:], pr[:, :, 1, :], sinb)
                nc.vector.tensor_mul(tb[:], pr[:, :, 0, :], cosb)
                nc.vector.tensor_sub(rot[:, :, 0, :], tb[:], ta[:])
                nc.vector.tensor_mul(ta[:], pr[:, :, 0, :], sinb)
                nc.vector.tensor_mul(tb[:], pr[:, :, 1, :], cosb)
                nc.vector.tensor_add(rot[:, :, 1, :], tb[:], ta[:])
                nc.sync.dma_start(
                    dram[tt * 128:(tt + 1) * 128, :],
                    rot[:].rearrange("p h two f -> p (h two f)"))

            for half in range(4):
                tts = range(half * (TT // 4), (half + 1) * (TT // 4))
                hsl = slice(half * (T // 4), (half + 1) * (T // 4))
                for tt in tts:
                    rsl = slice(tt * 128, (tt + 1) * 128)
                    xt = ln_pool.tile([128, HIDDEN], F32, tag="xt")
                    nc.sync.dma_start(xt[:], x[rsl, :])
                    stats = ln_pool.tile([128, 2, 6], F32, tag="st")
                    xr = xt[:].rearrange("p (s d) -> p s d", s=2)
                    for i in range(2):
                        nc.vector.bn_stats(out=stats[:, i, :], in_=xr[:, i, :])
                    mv = ln_pool.tile([128, 2], F32, tag="mv")
                    nc.vector.bn_aggr(out=mv[:], in_=stats[:])
                    std = ln_pool.tile([128, 1], F32, tag="sd")
                    nc.scalar.activation(out=std[:], in_=mv[:, 1:2], func=AF.Sqrt,
                                         bias=eps_sb[:])
                    rstd = ln_pool.tile([128, 1], F32, tag="rs")
                    nc.vector.reciprocal(out=rstd[:], in_=std[:])
                    xnb = ln_pool.tile([128, HIDDEN], BF16, tag="xnb")
                    if trivial_ln:
                        nc.vector.tensor_scalar(out=xnb[:], in0=xt[:],
                                                scalar1=mv[:, 0:1],
                                                scalar2=rstd[:],
                                                op0=ALU.subtract, op1=ALU.mult)
                    else:
                        nc.vector.tensor_scalar(out=xt[:], in0=xt[:],
                                                scalar1=mv[:, 0:1],
                                                scalar2=rstd[:],
                                                op0=ALU.subtract, op1=ALU.mult)
                        nc.gpsimd.tensor_mul(xt[:], xt[:], gamma_sb[:])
                        nc.vector.tensor_add(xnb[:], xt[:], beta_sb[:])
                    nc.sync.dma_start(xn_dram[rsl, :], xnb[:])
                    warm(2)
                for ks in range(KO):
                    nc.sync.dma_start_transpose(
                        xnT[:, ks, hsl],
                        xn_dram.ap()[hsl, ks * 128:(ks + 1) * 128])
                    warm(1)
                for tt in tts:
                    tsl = slice(tt * 128, (tt + 1) * 128)
                    psq = b1_ps.tile([128, QK], F32, tag="mq")
                    psk = b1_ps.tile([128, QK], F32, tag="mk")
                    psv = b1_ps.tile([128, QK], F32, tag="mv")
                    for ks in range(KO):
                        st, sp = (ks == 0), (ks == KO - 1)
                        nc.tensor.matmul(psq[:], xnT[:, ks, tsl], wt[:, ks, 0:QK],
                                         start=st, stop=sp)
                        nc.tensor.matmul(psk[:], xnT[:, ks, tsl], wt[:, ks, QK:2 * QK],
                                         start=st, stop=sp)
                        nc.tensor.matmul(psv[:], xnT[:, ks, tsl], wt[:, ks, 2 * QK:],
                                         start=st, stop=sp)
                    rotary_evict(psq, tt, q_dram)
                    rotary_evict(psk, tt, k_dram)
                    pv = psv[:].rearrange("p (h d) -> p h d", h=HG)
                    nc.vector.tensor_copy(out=v_aug[:, tt, :, 0:64], in_=pv)
                for hc in range(4):
                    nc.sync.dma_start_transpose(
                        qT[:, hc, hsl], q_dram.ap()[hsl, hc * 128:(hc + 1) * 128])
                    nc.sync.dma_start_transpose(
                        kT[:, hc, hsl], k_dram.ap()[hsl, hc * 128:(hc + 1) * 128])

        # phase-3 weight loads stream during phase 2
        nc.sync.dma_start(w_op_sb[:], w_op[:])
        nc.sync.dma_start(w_oo_sb[:], w_oo[:])
        nc.gpsimd.dma_start(out=bvec_sb[:], in_=bcast_ap(bvec.ap()))

        # ---- phase 2: p projection + GELU -> pt_dram ---------------------
        with tc.tile_pool(name="p2w", bufs=3) as p2w, \
             tc.tile_pool(name="p2g", bufs=3) as p2g, \
             tc.tile_pool(name="p2ps", bufs=2, space="PSUM") as p2ps:
            for pc in range(16):
                wpt = p2w.tile([128, KO, 128], BF16, tag="wp")
                nc.gpsimd.dma_start(out=wpt[:], in_=w_p.ap()[:, :, pc * 128:(pc + 1) * 128])
                for ic in range(NIC):
                    isl = slice(ic * 512, (ic + 1) * 512)
                    pp = p2ps.tile([128, 512], F32, tag="pp")
                    for ks in range(KO):
                        nc.tensor.matmul(pp[:], wpt[:, ks, :], xnT[:, ks, isl],
                                         start=(ks == 0), stop=(ks == KO - 1))
                    pt_sb = p2g.tile([128, 512], BF16, tag="pt")
                    nc.scalar.activation(pt_sb[:], pp[:], AF.Gelu)
                    nc.sync.dma_start(pt_dram.ap()[pc, :, isl], pt_sb[:])

        xnp_cm.__exit__(None, None, None)

        # ---- phase 3: attention with out_proj interleave -----------------
        with tc.tile_pool(name="pt3", bufs=1) as pt3, \
             tc.tile_pool(name="e3", bufs=3) as e3, \
             tc.tile_pool(name="n3", bufs=2) as n3, \
             tc.tile_pool(name="d3", bufs=2) as d3, \
             tc.tile_pool(name="s_ps", bufs=2, space="PSUM") as s_ps_pool, \
             tc.tile_pool(name="po_ps", bufs=1, space="PSUM") as po_ps_pool, \
             tc.tile_pool(name="o_ps", bufs=2, space="PSUM") as o_ps_pool:
            pt_a = pt3.tile([128, 16, 512], BF16)
            pt_b = pt3.tile([128, 16, 512], BF16)
            pt_tiles = [pt_a, pt_b]

            def ptload_closures(j):
                ptt = pt_tiles[j % 2]
                jsl = slice(j * 512, (j + 1) * 512)
                cls = []
                for pc in range(16):
                    def f(pc=pc, ptt=ptt, jsl=jsl):
                        nc.sync.dma_start(ptt[:, pc, :], pt_dram.ap()[pc, :, jsl])
                    cls.append(f)
                return cls

            def outproj_closures(j):
                ptt = pt_tiles[j % 2]
                state = {}
                cls = []
                for isub in range(4):
                    tok0 = j * 512 + isub * 128
                    ssl = slice(isub * 128, (isub + 1) * 128)
                    for oc in range(2):
                        osl = slice(oc * 512, (oc + 1) * 512)
                        for pc in range(16):
                            def f(pc=pc, ssl=ssl, osl=osl, ptt=ptt):
                                if pc == 0:
                                    state['po2'] = o_ps_pool.tile(
                                        [128, 512], F32, tag="po2",
                                        name="po2")
                                nc.tensor.matmul(state['po2'][:], ptt[:, pc, ssl],
                                                 w_op_sb[:, pc, osl],
                                                 start=(pc == 0), stop=False)
                            cls.append(f)
                        for ks in range(4):
                            def f(ks=ks, tok0=tok0, osl=osl):
                                nc.tensor.matmul(state['po2'][:],
                                                 oT[:, ks, tok0:tok0 + 128],
                                                 w_oo_sb[:, ks, osl],
                                                 start=False, stop=(ks == 3))
                            cls.append(f)
                        def f(oc=oc, osl=osl, tok0=tok0):
                            if oc == 0:
                                state['fin'] = d3.tile([128, HIDDEN], F32,
                                                       tag="fin", name="fin")
                            nc.vector.tensor_add(state['fin'][:, osl],
                                                 state['po2'][:], bvec_sb[:, osl])
                            if oc == 1:
                                nc.sync.dma_start(out.ap()[tok0:tok0 + 128, :],
                                                  state['fin'][:])
                        cls.append(f)
                return cls

            for ic in range(NIC):
                isl = slice(ic * 512, (ic + 1) * 512)
                fillers = ptload_closures(ic)
                if ic > 0:
                    fillers += outproj_closures(ic - 1)
                fi = 0
                for pr in range(4):
                    po_pair = po_ps_pool.tile([65, 1024], F32, tag="po")
                    prev_e = None
                    for jc in range(JC):
                        jsl = slice(jc * 128, (jc + 1) * 128)
                        s_pair = s_ps_pool.tile([128, 1024], F32, tag="s")
                        nc.tensor.matmul(s_pair[:, 0:512], kT[0:64, pr, jsl],
                                         qT[0:64, pr, isl], start=True, stop=True)
                        nc.tensor.matmul(s_pair[:, 512:1024], kT[64:128, pr, jsl],
                                         qT[64:128, pr, isl], start=True, stop=True)
                        e_pair = e3.tile([128, 1024], BF16, tag="e")
                        nc.scalar.activation(e_pair[:], s_pair[:], AF.Exp,
                                             scale=0.125)
                        for _ in range(3):
                            if fi < len(fillers):
                                fillers[fi]()
                                fi += 1
                        if prev_e is not None:
                            jm = jc - 1
                            nc.tensor.matmul(po_pair[:, 0:512],
                                             v_aug[:, jm, 2 * pr, :],
                                             prev_e[:, 0:512],
                                             start=(jm == 0), stop=False)
                            nc.tensor.matmul(po_pair[:, 512:1024],
                                             v_aug[:, jm, 2 * pr + 1, :],
                                             prev_e[:, 512:1024],
                                             start=(jm == 0), stop=False)
                        prev_e = e_pair
                    nc.tensor.matmul(po_pair[:, 0:512], v_aug[:, 15, 2 * pr, :],
                                     prev_e[:, 0:512], start=False, stop=True)
                    nc.tensor.matmul(po_pair[:, 512:1024],
                                     v_aug[:, 15, 2 * pr + 1, :],
                                     prev_e[:, 512:1024], start=False, stop=True)
                    po_sb = n3.tile([65, 1024], F32, tag="posb")
                    nc.vector.tensor_copy(out=po_sb[:], in_=po_pair[:])
                    rz = n3.tile([1, 1024], F32, tag="rz")
                    nc.vector.tensor_copy(out=rz[:], in_=po_sb[64:65, :])
                    rzb = n3.tile([64, 1024], F32, tag="rzb")
                    nc.gpsimd.partition_broadcast(rzb[:], rz[:])
                    nc.vector.reciprocal_approx_fast(rzb[:], rzb[:])
                    nc.vector.tensor_mul(oT[0:64, pr, isl], po_sb[0:64, 0:512],
                                         rzb[:, 0:512])
                    nc.vector.tensor_mul(oT[64:128, pr, isl],
                                         po_sb[0:64, 512:1024],
                                         rzb[:, 512:1024])
                while fi < len(fillers):
                    fillers[fi]()
                    fi += 1

            # ---- phase 4: out_proj tail for the last window --------------
            for f in outproj_closures(NIC - 1):
                f()

        if debug:
            nc.sync.dma_start(ot_dbg.ap(), oT[:])
            nc.sync.dma_start(qt_dbg.ap(), qT[:])
            nc.sync.dma_start(kt_dbg.ap(), kT[:])
            nc.sync.dma_start(va_dbg.ap(), v_aug[:])
            nc.sync.dma_start(pt_dbg.ap(), pt_dram.ap())
        p3w_cm.__exit__(None, None, None)
        persist_cm.__exit__(None, None, None)

    nc.compile()
    return nc


_NC_CACHE = {}


def _get_nc(trivial_ln):
    if trivial_ln not in _NC_CACHE:
        _NC_CACHE[trivial_ln] = _build_nc(trivial_ln)
    return _NC_CACHE[trivial_ln]


def _host_tables():
    inv_freq = 1.0 / (10000.0 ** (np.arange(0, HEAD_DIM, 2, dtype=np.float32)
                                  / HEAD_DIM))
    ang = np.arange(T, dtype=np.float32)[:, None] * inv_freq[None, :]
    return np.cos(ang).astype(np.float32), np.sin(ang).astype(np.float32)


def _shard_weights(w_in, w_out, b_out, ln_gamma, ln_beta, x):
    cos_np, sin_np = _host_tables()
    bf = ml_dtypes.bfloat16

    def fold(a, ko):
        # [ko*128, c] -> [128, ko, c] with [p, k, c] = a[k*128 + p, c]
        return np.ascontiguousarray(
            a.reshape(ko, 128, a.shape[1]).transpose(1, 0, 2))

    in_maps = []
    for c in range(N_CORES):
        b, g = c // 2, c % 2
        sl = slice(g * QK, (g + 1) * QK)
        w_qkv = np.concatenate(
            [w_in[:, 0 * HIDDEN:][:, sl], w_in[:, 1 * HIDDEN:][:, sl],
             w_in[:, 2 * HIDDEN:][:, sl]], axis=1)
        w_p = w_in[:, 3 * HIDDEN + g * PCOLS:3 * HIDDEN + (g + 1) * PCOLS]
        w_oo = w_out[g * QK:(g + 1) * QK, :]
        w_op = w_out[HIDDEN + g * PCOLS:HIDDEN + (g + 1) * PCOLS, :]
        in_maps.append({
            "x": np.ascontiguousarray(x[b]).astype(np.float32),
            "gamma": ln_gamma.astype(np.float32),
            "beta": ln_beta.astype(np.float32),
            "w_qkv": fold(w_qkv, KO).astype(bf),
            "w_p": fold(w_p, KO).astype(bf),
            "w_oo": fold(w_oo, 4).astype(bf),
            "w_op": fold(w_op, 16).astype(bf),
            "bvec": (b_out if g == 0 else np.zeros_like(b_out)).astype(np.float32),
            "cos_t": cos_np,
            "sin_t": sin_np,
        })
    return in_maps


def kernel(x, ln_gamma, ln_beta, w_in, w_out, b_out, _trace=False, _tmpdir=None):
    x = np.asarray(x, dtype=np.float32)
    ln_gamma = np.asarray(ln_gamma, dtype=np.float32)
    ln_beta = np.asarray(ln_beta, dtype=np.float32)
    w_in = np.asarray(w_in, dtype=np.float32)
    w_out = np.asarray(w_out, dtype=np.float32)
    b_out = np.asarray(b_out, dtype=np.float32)

    trivial_ln = bool(np.allclose(ln_gamma, 1.0) and np.allclose(ln_beta, 0.0))
    nc = _get_nc(trivial_ln)
    in_maps = _shard_weights(w_in, w_out, b_out, ln_gamma, ln_beta, x)
    kwargs = {}
    if _trace:
        kwargs = {"trace": True, "tmpdir": _tmpdir}
    res = None
    last_err = None
    for _attempt in range(3):
        try:
            res = run_bass_kernel_spmd(nc, in_maps,
                                       core_ids=list(range(N_CORES)), **kwargs)
            break
        except Exception as e:  # transient device flakes (NRT_EXEC_UNIT_...)
            last_err = e
    if res is None:
        raise last_err
    outs = [res.results[c]["out"] for c in range(N_CORES)]
    full = np.stack([outs[2 * b] + outs[2 * b + 1] for b in range(B)], axis=0)
    kernel._last_exec_time_ns = res.exec_time_ns
    return full.astype(np.float32)
